# revision 1
# baseline (speedup 1.0000x reference)
"""Trainium2 Bass kernel for 16-head causal MultiHeadAttention.

Problem: B=2, S=2048, D=1024, H=16 heads of 64. Causal mask, softmax,
fp32 weights/activations.

Sharding: tensor-parallel over heads. Each of the 8 cores handles 2 heads
(a 128-wide feature slice): it computes Q/K/V projections for its slice,
causal attention for its 2 heads over both batch elements, and a partial
output projection y_c = A_c @ Wo[c*128:(c+1)*128, :]. The host sums the 8
partials and adds bo (the "unshard" step).

Device layout notes (everything transposed, feature-on-partition):
  xT   [128, 8, 4096]   xT[p, kc, t] = x[t, kc*128+p]
  Q^T  [128, 4096]      rows = 2 heads x 64 feats, cols = token (b*2048+s)
  K^T  same
  V    [128, 32, 130]   normal layout: partition = token-within-tile,
                        per token tile: [V_h0(64) | ones | V_h1(64) | ones]
                        (ones column turns the PV matmul into a fused
                        attn+rowsum computation)
  S^T  [keys, queries]  per (b, head, 512-query chunk), computed per
                        128-key tile; exp (scale=1/8, no max subtraction:
                        scores are ~N(0,1) so exp never overflows; masked
                        entries are multiplied by 0 afterwards, matching
                        the reference's -10000 masking whose exp
                        underflows to 0 in fp32)
"""

import os
import sys
from contextlib import ExitStack

import numpy as np

for _p in ("/opt/trn_rl_repo",):
    if _p not in sys.path and os.path.isdir(_p):
        sys.path.insert(0, _p)

import concourse.bass as bass
import concourse.bacc as bacc
import concourse.tile as tile
from concourse import mybir
from concourse.bass import ts
from concourse.bass_utils import run_bass_kernel_spmd
from concourse.masks import make_identity

F32 = mybir.dt.float32
AF = mybir.ActivationFunctionType

B, S, D, H, HD = 2, 2048, 1024, 16, 64
T = B * S                     # 4096 tokens
NCORES = 8
FPC = D // NCORES             # 128 features per core (2 heads)
HPC = FPC // HD               # 2 heads per core
KC = D // 128                 # 8 contraction chunks for projections
TCH = T // 512                # 8 token chunks of 512
QCH = S // 512                # 4 query chunks per batch
NTT = T // 128                # 32 token tiles of 128

USE_F32R = os.environ.get("MHA_F32R", "1") == "1"
DEBUG_DUMP = os.environ.get("MHA_DEBUG", "0") == "1"


def build_nc(use_f32r: bool = USE_F32R, debug: bool = DEBUG_DUMP) -> bass.Bass:
    nc = bacc.Bacc()

    MM = mybir.dt.float32r if use_f32r else F32
    xT = nc.declare_dram_parameter("xT", [128, KC, T], MM, False)
    wq = nc.declare_dram_parameter("wq", [128, KC, FPC], MM, False)
    wk = nc.declare_dram_parameter("wk", [128, KC, FPC], MM, False)
    wv = nc.declare_dram_parameter("wv", [128, KC, FPC], MM, False)
    wo = nc.declare_dram_parameter("wo", [FPC, D], MM, False)
    bq = nc.declare_dram_parameter("bq", [FPC, 1], F32, False)
    bk = nc.declare_dram_parameter("bk", [FPC, 1], F32, False)
    bv = nc.declare_dram_parameter("bv", [FPC, 1], F32, False)
    maskT = nc.declare_dram_parameter("maskT", [128, 4, 512], MM, False)
    vpad = nc.declare_dram_parameter("vpad", [128, NTT, 2, HD], MM, False)
    qzero = nc.declare_dram_parameter("qzero", [HD, T], MM, False)
    yT = nc.declare_dram_parameter("yT", [D, T], F32, True)
    if debug:
        MMd = mybir.dt.float32r if use_f32r else F32
        dbg_qt = nc.declare_dram_parameter("dbg_qt", [128, T], MMd, True)
        dbg_kt = nc.declare_dram_parameter("dbg_kt", [128, T], MMd, True)
        dbg_v = nc.declare_dram_parameter("dbg_v", [128, NTT, 2 * (HD + 1)], MMd, True)
        dbg_at = nc.declare_dram_parameter("dbg_at", [128, T], MMd, True)
        dbg_acc = nc.declare_dram_parameter("dbg_acc", [65, 512], F32, True)
        dbg_rmat = nc.declare_dram_parameter("dbg_rmat", [64, 512], F32, True)

    with tile.TileContext(nc) as tc, ExitStack() as ctx:
        const = ctx.enter_context(tc.tile_pool(name="const", bufs=1))
        persist = ctx.enter_context(tc.tile_pool(name="persist", bufs=1))
        xt_pool = ctx.enter_context(tc.tile_pool(name="xt_pool", bufs=2))
        vt_pool = ctx.enter_context(tc.tile_pool(name="vt_pool", bufs=2))
        pt_pool = ctx.enter_context(tc.tile_pool(name="pt_pool", bufs=4))
        yt_pool = ctx.enter_context(tc.tile_pool(name="yt_pool", bufs=3))
        rmat_pool = ctx.enter_context(tc.tile_pool(name="rmat_pool", bufs=3))
        recip_pool = ctx.enter_context(tc.tile_pool(name="recip_pool", bufs=3))
        tmp_pool = ctx.enter_context(tc.tile_pool(name="tmp_pool", bufs=2))
        accsb_pool = ctx.enter_context(tc.tile_pool(name="accsb_pool", bufs=3))

        wq_sb = const.tile([128, KC, FPC], MM)
        wk_sb = const.tile([128, KC, FPC], MM)
        wv_sb = const.tile([128, KC, FPC], MM)
        wo_sb = const.tile([FPC, D], MM)
        bq_sb = const.tile([FPC, 1], F32)
        bk_sb = const.tile([FPC, 1], F32)
        bv_sb = const.tile([FPC, 1], F32)
        mask_sb = const.tile([128, 4, 512], MM)
        ident = const.tile([128, 128], F32)
        shid = const.tile([128, 128], F32)
        ones65 = const.tile([65, 128], F32)
        tmp2 = const.tile([128, 512], F32)
        # weight/bias DMAs first: the projection pipeline only needs these
        # plus the first xT chunk, so everything else loads in its shadow
        nc.scalar.dma_start(out=wq_sb, in_=wq[:])
        nc.scalar.dma_start(out=wk_sb, in_=wk[:])
        nc.scalar.dma_start(out=wv_sb, in_=wv[:])
        nc.scalar.dma_start(out=bq_sb, in_=bq[:])
        nc.scalar.dma_start(out=bk_sb, in_=bk[:])
        nc.scalar.dma_start(out=bv_sb, in_=bv[:])
        make_identity(nc, ident)
        nc.gpsimd.memset(shid, 0.0)
        nc.gpsimd.affine_select(
            out=shid,
            in_=shid,
            compare_op=mybir.AluOpType.not_equal,
            fill=1.0,
            base=64,
            pattern=[[-1, 128]],
            channel_multiplier=1,
        )
        nc.vector.memset(ones65, 1.0)
        nc.vector.memset(tmp2, 0.0)

        QT0z = persist.tile([128, T], MM)
        QT1z = persist.tile([128, T], MM)
        KT = persist.tile([128, T], MM)
        V = persist.tile([128, NTT, 2 * 128], MM)
        AT = persist.tile([128, T], MM)
        vslots = V.rearrange("p t (g x) -> p t g x", g=2)  # x = 128
        # cols 64.. of each head slot: [1, 0, 0, ...] -> fused attn+rowsum with
        # a full-128-column stationary operand (keeps the PE array fully lit)
        nc.scalar.dma_start(out=vslots[:, :, :, HD:], in_=vpad[:])
        # zero the unused head half of each padded Q so S^T can contract k=128
        nc.scalar.dma_start(out=QT0z[HD:, :], in_=qzero[:])
        nc.scalar.dma_start(out=QT1z[0:HD, :], in_=qzero[:])
        nc.scalar.dma_start(out=wo_sb, in_=wo[:])
        nc.scalar.dma_start(out=mask_sb, in_=maskT[:])

        # ---- projections: Q^T, K^T (feature-major) and V (token-major) ----
        with (
            tc.tile_pool(name="proj_ps", bufs=4, space="PSUM") as proj_ps,
            tc.tile_pool(name="tr_ps", bufs=2, space="PSUM") as tr_ps,
        ):
            for tcn in range(TCH):
                xt = xt_pool.tile([128, KC, 512], MM)
                for kc in range(KC):
                    nc.sync.dma_start(
                        out=xt[:, kc, :], in_=xT[:, kc, ts(tcn, 512)]
                    )
                # V first: its chain (matmuls -> copy -> transposes -> DVE
                # copies) is the deepest, so start it earliest
                ps = proj_ps.tile([128, 512], F32, name="proj_psum")
                for kc in range(KC):
                    nc.tensor.matmul(
                        ps,
                        wv_sb[:, kc, :],
                        xt[:, kc, :],
                        start=(kc == 0),
                        stop=(kc == KC - 1),
                    )
                vt = vt_pool.tile([128, 512], F32)
                nc.scalar.activation(vt, ps, AF.Identity, bias=bv_sb)
                for i in range(4):
                    tp = tr_ps.tile([128, 128], F32, name="tr_psum")
                    nc.tensor.transpose(tp, vt[:, ts(i, 128)], ident)
                    tt = tcn * 4 + i
                    nc.vector.tensor_copy(
                        vslots[:, tt, :, 0:HD],
                        tp.rearrange("p (g f) -> p g f", g=2),
                    )
                for wsb, bsb, dests in (
                    (wk_sb, bk_sb, KT),
                    (wq_sb, bq_sb, None),
                ):
                    ps = proj_ps.tile([128, 512], F32, name="proj_psum")
                    for kc in range(KC):
                        nc.tensor.matmul(
                            ps,
                            wsb[:, kc, :],
                            xt[:, kc, :],
                            start=(kc == 0),
                            stop=(kc == KC - 1),
                        )
                    if dests is None:
                        nc.scalar.activation(
                            QT0z[0:HD, ts(tcn, 512)],
                            ps[0:HD, :],
                            AF.Identity,
                            bias=bq_sb[0:HD],
                        )
                        nc.scalar.activation(
                            QT1z[HD:, ts(tcn, 512)],
                            ps[HD:, :],
                            AF.Identity,
                            bias=bq_sb[HD:],
                        )
                    else:
                        nc.scalar.activation(
                            dests[:, ts(tcn, 512)], ps, AF.Identity, bias=bsb
                        )

        # ---- attention + interleaved output projection ----
        # Heads are processed together per key tile: the two k=64 S^T matmuls
        # go to row groups 0-1 / 2-3 via tile_position auto-derivation and run
        # concurrently in the PE array (also keeps full-array activity so the
        # HAM clock gate stays open).
        with (
            tc.tile_pool(name="st_ps", bufs=2, space="PSUM") as st_ps,
            tc.tile_pool(name="acc_ps", bufs=2, space="PSUM") as acc_ps,
            tc.tile_pool(name="op_ps", bufs=2, space="PSUM") as op_ps,
        ):
            def outproj(g0):
                for mt in range(D // 128):
                    ps = op_ps.tile([128, 512], F32, name="op_psum")
                    nc.tensor.matmul(
                        ps,
                        wo_sb[:, ts(mt, 128)],
                        AT[:, g0 : g0 + 512],
                        start=True,
                        stop=True,
                    )
                    yt = yt_pool.tile([128, 512], F32, name="yt")
                    nc.vector.tensor_copy(yt, ps)
                    nc.sync.dma_start(out=yT[ts(mt, 128), g0 : g0 + 512], in_=yt)

            pending_g0 = None
            for b in range(B):
                for qc in range(QCH):
                    g0 = b * S + qc * 512
                    nkt = 4 * (qc + 1)  # causal: number of 128-key tiles
                    acc0 = acc_ps.tile([128, 512], F32, name="accp", tag="accp")
                    acc1 = acc_ps.tile([128, 512], F32, name="accp", tag="accp")
                    accs = (acc0, acc1)
                    # process the masked diagonal tiles first: their longer
                    # exp -> mask -> PV chains overlap the unmasked tiles that
                    # follow instead of delaying the normalize chain at the tail
                    kt_order = list(range(4 * qc, nkt)) + list(range(0, 4 * qc))
                    for ktpos, kt in enumerate(kt_order):
                        k0 = b * S + kt * 128
                        st = st_ps.tile([128, 1024], F32, name="st_psum")
                        for hl, qtz in ((0, QT0z), (1, QT1z)):
                            nc.tensor.matmul(
                                st[:, ts(hl, 512)],
                                KT[:, k0 : k0 + 128],
                                qtz[:, g0 : g0 + 512],
                                start=True,
                                stop=True,
                            )
                        pt = pt_pool.tile([128, 1024], MM)
                        nc.scalar.activation(pt, st, AF.Exp, scale=0.125)
                        d = kt - 4 * qc
                        if d >= 0:
                            m = mask_sb[:, d, :]
                            m2 = bass.AP(
                                tensor=m.tensor,
                                offset=m.offset,
                                ap=[list(m.ap[0]), [0, 2], list(m.ap[1])],
                            )
                            nc.gpsimd.tensor_mul(
                                pt.rearrange("p (h j) -> p h j", h=2), pt.rearrange("p (h j) -> p h j", h=2), m2
                            )
                        for hl in range(HPC):
                            nc.tensor.matmul(
                                accs[hl],
                                vslots[:, b * (S // 128) + kt, hl, :],
                                pt[:, ts(hl, 512)],
                                start=(ktpos == 0),
                                stop=(ktpos == nkt - 1),
                            )
                    for hl in (1, 0):
                        acc = accs[hl]
                        # drain acc to SBUF immediately so the PSUM bank
                        # frees for the next chunk's PV accumulation
                        acc_sb = accsb_pool.tile([HD + 1, 512], F32)
                        nc.vector.tensor_copy(acc_sb, acc[0 : HD + 1, :])
                        recip = recip_pool.tile([HD + 1, 512], F32)
                        nc.vector.reciprocal(
                            recip[HD : HD + 1, :], acc_sb[HD : HD + 1, :]
                        )
                        rmat_ps = op_ps.tile([128, 512], F32, name="op_psum")
                        nc.tensor.matmul(
                            rmat_ps,
                            ones65[HD : HD + 1, :],
                            recip[HD : HD + 1, :],
                            start=True,
                            stop=True,
                        )
                        rmat = rmat_pool.tile([HD, 512], F32)
                        nc.scalar.activation(rmat, rmat_ps[0:HD, :], AF.Copy)
                        if debug and b == 0 and qc == 0 and hl == 0:
                            nc.sync.dma_start(out=dbg_acc[:], in_=acc_sb)
                            nc.sync.dma_start(out=dbg_rmat[:], in_=rmat)
                        if hl == 0:
                            nc.vector.tensor_mul(
                                AT[0:HD, g0 : g0 + 512], acc_sb[0:HD, :], rmat
                            )
                        else:
                            nc.vector.tensor_mul(
                                tmp2[0:HD, :], acc_sb[0:HD, :], rmat
                            )
                            sh = op_ps.tile([128, 512], F32, name="op_psum")
                            nc.tensor.matmul(
                                sh,
                                shid,
                                tmp2,
                                start=True,
                                stop=True,
                            )
                            nc.scalar.activation(
                                AT[HD : 2 * HD, g0 : g0 + 512],
                                sh[HD : 2 * HD, :],
                                AF.Copy,
                            )
                    # partial output projection, deferred one chunk so the
                    # tensor engine never drains while the normalize chain runs
                    if pending_g0 is not None:
                        outproj(pending_g0)
                    pending_g0 = g0
            outproj(pending_g0)
        if debug:
            nc.sync.dma_start(out=dbg_qt[:], in_=QT)
            nc.sync.dma_start(out=dbg_kt[:], in_=KT)
            nc.sync.dma_start(out=dbg_v[:], in_=V)
            nc.sync.dma_start(out=dbg_at[:], in_=AT)

    nc.finalize()
    return nc


def _install_ntff_hook():
    """bass_utils' trace path needs antenv.axon_hooks, which this image's
    antenv lacks; synthesize it from the boot helper so NTFF profiling works."""
    try:
        from antenv.axon_hooks import get_axon_ntff_profile_hook  # noqa: F401

        return
    except ImportError:
        pass
    try:
        import types

        import antenv
        from trn_agent_boot.trn_boot import _ntff_profile_via_ctypes

        hook = _ntff_profile_via_ctypes("/opt/axon/libaxon_pjrt.so")
        mod = types.ModuleType("antenv.axon_hooks")
        state = {"hook": hook}
        mod.get_axon_ntff_profile_hook = lambda: state["hook"]
        mod.set_axon_ntff_profile_hook = lambda h: state.update(hook=h)
        sys.modules["antenv.axon_hooks"] = mod
        antenv.axon_hooks = mod
    except Exception:
        pass


_NC_CACHE: dict[bool, bass.Bass] = {}


def _get_nc(use_f32r: bool) -> bass.Bass:
    if use_f32r not in _NC_CACHE:
        _NC_CACHE[use_f32r] = build_nc(use_f32r)
    return _NC_CACHE[use_f32r]


def _shard_inputs(inputs, Wq, bq, Wk, bk, Wv, bv, Wo, bo):
    x = np.ascontiguousarray(np.asarray(inputs, dtype=np.float32)).reshape(T, D)
    # xT[p, kc, t] = x[t, kc*128+p]
    xTh = np.ascontiguousarray(x.reshape(T, KC, 128).transpose(2, 1, 0))

    maskh = np.zeros((128, 4, 512), dtype=np.float32)
    p = np.arange(128)[:, None]
    jj = np.arange(512)[None, :]
    for d in range(4):
        maskh[:, d, :] = (d * 128 + p <= jj).astype(np.float32)

    def wslice(W, c):
        Wc = np.asarray(W, dtype=np.float32)[:, c * FPC : (c + 1) * FPC]
        # [128, KC, FPC] with [p, kc, m] = W[kc*128+p, m]
        return np.ascontiguousarray(Wc.reshape(KC, 128, FPC).transpose(1, 0, 2))

    vpad_h = np.zeros((128, NTT, 2, HD), dtype=np.float32)
    vpad_h[:, :, :, 0] = 1.0
    in_maps = []
    for c in range(NCORES):
        in_maps.append(
            {
                "xT": xTh,
                "wq": wslice(Wq, c),
                "wk": wslice(Wk, c),
                "wv": wslice(Wv, c),
                "wo": np.ascontiguousarray(
                    np.asarray(Wo, dtype=np.float32)[c * FPC : (c + 1) * FPC, :]
                ),
                "bq": np.asarray(bq, np.float32)[c * FPC : (c + 1) * FPC, None],
                "bk": np.asarray(bk, np.float32)[c * FPC : (c + 1) * FPC, None],
                "bv": np.asarray(bv, np.float32)[c * FPC : (c + 1) * FPC, None],
                "maskT": maskh,
                "vpad": vpad_h,
                "qzero": np.zeros((HD, T), dtype=np.float32),
            }
        )
    return in_maps


def run_with_results(
    inputs,
    Wq,
    bq,
    Wk,
    bk,
    Wv,
    bv,
    Wo,
    bo,
    trace: bool = False,
    use_f32r: bool = USE_F32R,
):
    in_maps = _shard_inputs(inputs, Wq, bq, Wk, bk, Wv, bv, Wo, bo)
    if trace:
        _install_ntff_hook()
    nc = _get_nc(use_f32r)
    res = run_bass_kernel_spmd(
        nc, in_maps, core_ids=list(range(NCORES)), trace=trace
    )
    acc = np.zeros((D, T), dtype=np.float32)
    for c in range(NCORES):
        acc += res.results[c]["yT"]
    y = acc.T + np.asarray(bo, np.float32)[None, :]
    out = np.ascontiguousarray(y.reshape(B, S, D).astype(np.float32))
    return out, res


def kernel(**inputs) -> np.ndarray:
    out, _ = run_with_results(**inputs)
    return out


if __name__ == "__main__":
    nc = build_nc()
    print("built ok")



# revision 13
# speedup vs baseline: 1.6905x; 1.6905x over previous
"""Trainium2 Bass kernel for 16-head causal MultiHeadAttention.

Problem: B=2, S=2048, D=1024, H=16 heads of 64. Causal mask, softmax,
fp32 reference (computed in bf16 on the PE, fp32 PSUM accum; rel-err
budget 2e-2, measured ~6e-3).

Sharding: tensor-parallel over heads. Each of the 8 cores handles 2 heads
(a 128-wide feature slice): it computes Q/K/V projections for its slice,
causal attention for its 2 heads over both batch elements, and a partial
output projection y_c = A_c @ Wo[c*128:(c+1)*128, :]. The host sums the 8
partials and adds bo (the "unshard" step).

Schedule: projection token-chunks (PE-heavy) are interleaved with
attention query-chunks (scalar/exp-heavy) - P0 P1 A00 P2 A01 ... - so the
two phases overlap instead of running back to back.

Device layout notes (everything transposed, feature-on-partition):
  xT   [128, 8, 4096]   xT[p, kc, t] = x[t, kc*128+p]            (bf16)
  Q^T  [128, 4096]      rows = 2 heads x 64 feats, cols = token
  K^T  same. Scores are k=64 matmuls on the head's 64-partition slice;
                        the two heads auto-derive tile_position row
                        groups 0/64 and run concurrently in the PE array
  V    [128, 32, 2, 128] token-major slots per 128-token tile:
                        slot0 = [V_h0(64) | ones | 0...]  -> acc0 rows
                        0:64 = attn_h0, row 64 = rowsum_h0
                        slot1 = [ones | 0... | V_h1(64 @ cols 64:128)]
                        -> acc1 row 0 = rowsum_h1, rows 64:128 = attn_h1
                        (h1 lands directly on partitions 64:127, so no
                        partition-shift matmul is needed to assemble A^T)
  S^T  [keys, queries]  per (b, head, 512-query chunk), computed per
                        128-key tile; exp (scale=1/8, no max subtraction:
                        scores are ~N(0,1) so exp never overflows; masked
                        entries are multiplied by 0 afterwards, matching
                        the reference's -10000 masking whose exp
                        underflows to 0)
  normalize: both rowsums gathered into one SBUF tile, broadcast to all
  128 partitions with one k=128 selector matmul, then one full-width DVE
  reciprocal_approx_fast (~18 good bits; the custom op only honors base
  partition 0, hence recip-after-broadcast) and two DVE multiplies.
"""

import os
import sys
from contextlib import ExitStack

import numpy as np

for _p in ("/opt/trn_rl_repo",):
    if _p not in sys.path and os.path.isdir(_p):
        sys.path.insert(0, _p)

import ml_dtypes

import concourse.bass as bass
import concourse.bacc as bacc
import concourse.tile as tile
from concourse import mybir
from concourse.bass import ts
from concourse.bass_utils import run_bass_kernel_spmd
from concourse.masks import make_identity

F32 = mybir.dt.float32
BF = mybir.dt.bfloat16
AF = mybir.ActivationFunctionType
BF_NP = ml_dtypes.bfloat16

B, S, D, H, HD = 2, 2048, 1024, 16, 64
T = B * S                     # 4096 tokens
NCORES = 8
FPC = D // NCORES             # 128 features per core (2 heads)
HPC = FPC // HD               # 2 heads per core
KC = D // 128                 # 8 contraction chunks for projections
TCH = T // 512                # 8 token chunks of 512
QCH = S // 512                # 4 query chunks per batch
NTT = T // 128                # 32 token tiles of 128


def build_nc() -> bass.Bass:
    nc = bacc.Bacc()

    xT = nc.declare_dram_parameter("xT", [128, KC, T], BF, False)
    wq = nc.declare_dram_parameter("wq", [128, KC, FPC], BF, False)
    wk = nc.declare_dram_parameter("wk", [128, KC, FPC], BF, False)
    wv = nc.declare_dram_parameter("wv", [128, KC, FPC], BF, False)
    wo = nc.declare_dram_parameter("wo", [FPC, D], BF, False)
    bq = nc.declare_dram_parameter("bq", [FPC, 1], F32, False)
    bk = nc.declare_dram_parameter("bk", [FPC, 1], F32, False)
    bv = nc.declare_dram_parameter("bv", [FPC, 1], F32, False)
    maskT = nc.declare_dram_parameter("maskT", [128, 4, 512], BF, False)
    yT = nc.declare_dram_parameter("yT", [D, T], BF, True)

    with tile.TileContext(nc) as tc, ExitStack() as ctx:
        const = ctx.enter_context(tc.tile_pool(name="const", bufs=1))
        persist = ctx.enter_context(tc.tile_pool(name="persist", bufs=1))
        xt_pool = ctx.enter_context(tc.tile_pool(name="xt_pool", bufs=2))
        vt_pool = ctx.enter_context(tc.tile_pool(name="vt_pool", bufs=2))
        pt_pool = ctx.enter_context(tc.tile_pool(name="pt_pool", bufs=4))
        yt_pool = ctx.enter_context(tc.tile_pool(name="yt_pool", bufs=3))
        rmat_pool = ctx.enter_context(tc.tile_pool(name="rmat_pool", bufs=2))

        wq_sb = const.tile([128, KC, FPC], BF)
        wk_sb = const.tile([128, KC, FPC], BF)
        wv_sb = const.tile([128, KC, FPC], BF)
        wo_sb = const.tile([FPC, D], BF)
        bq_sb = const.tile([FPC, 1], F32)
        bk_sb = const.tile([FPC, 1], F32)
        bv_sb = const.tile([FPC, 1], F32)
        mask_sb = const.tile([128, 4, 512], BF)
        ident = const.tile([128, 128], BF)
        sel2 = const.tile([128, 128], F32)
        # V-chain consts first: the first projection only needs wv/bv plus
        # the first xT chunk, so everything else loads in their shadow
        nc.scalar.dma_start(out=wv_sb, in_=wv[:])
        nc.scalar.dma_start(out=bv_sb, in_=bv[:])
        nc.scalar.dma_start(out=wk_sb, in_=wk[:])
        nc.scalar.dma_start(out=bk_sb, in_=bk[:])
        nc.scalar.dma_start(out=wq_sb, in_=wq[:])
        nc.scalar.dma_start(out=bq_sb, in_=bq[:])
        make_identity(nc, ident)
        # selector for the rowsum broadcast: rmat_ps = sel2^T @ rsrc puts
        # rsrc row 64 (rowsum_h0) on partitions 0:64 and rsrc row 0
        # (rowsum_h1) on partitions 64:128
        nc.vector.memset(sel2, 0.0)
        nc.vector.memset(sel2[HD : HD + 1, 0:HD], 1.0)
        nc.vector.memset(sel2[0:1, HD:128], 1.0)

        QT = persist.tile([128, T], BF)
        KT = persist.tile([128, T], BF)
        V = persist.tile([128, NTT, 2, 128], BF)
        AT = persist.tile([128, T], BF)
        rsrc = persist.tile([128, 512], F32)
        vslots = V  # [128, NTT, 2, 128]
        # V slot constants: zeros on the halves the V copies won't write,
        # ones columns that turn the PV matmul into fused attn+rowsum
        nc.gpsimd.memset(vslots[:, :, 0, HD:128], 0.0)
        nc.gpsimd.memset(vslots[:, :, 1, 0:HD], 0.0)
        nc.gpsimd.memset(vslots[:, :, 0, HD : HD + 1], 1.0)
        nc.gpsimd.memset(vslots[:, :, 1, 0:1], 1.0)
        # rsrc rows other than 0/64 are contracted by sel2 zeros but must
        # be finite
        nc.gpsimd.memset(rsrc, 0.0)
        nc.scalar.dma_start(out=wo_sb, in_=wo[:])
        nc.scalar.dma_start(out=mask_sb, in_=maskT[:])

        with (
            tc.tile_pool(name="st_ps", bufs=2, space="PSUM") as st_ps,
            tc.tile_pool(name="acc_ps", bufs=2, space="PSUM") as acc_ps,
            tc.tile_pool(name="mix_ps", bufs=2, space="PSUM") as mix_ps,
        ):
            def emit_proj(tcn):
                xts = []
                for kc in range(KC):
                    xt = xt_pool.tile([128, 512], BF, name=f"xt{kc}")
                    nc.sync.dma_start(out=xt, in_=xT[:, kc, ts(tcn, 512)])
                    xts.append(xt)
                # V first: its chain (matmuls -> copy -> transposes -> DVE
                # copies) is the deepest, so start it earliest
                ps = mix_ps.tile([128, 512], F32, name="mix_psum")
                for kc in range(KC):
                    nc.tensor.matmul(
                        ps,
                        wv_sb[:, kc, :],
                        xts[kc],
                        start=(kc == 0),
                        stop=(kc == KC - 1),
                    )
                vt = vt_pool.tile([128, 512], BF)
                nc.scalar.activation(vt, ps, AF.Identity, bias=bv_sb)
                for i in range(4):
                    tp = mix_ps.tile([128, 128], BF, name="mix_psum")
                    nc.tensor.transpose(tp, vt[:, ts(i, 128)], ident)
                    tt = tcn * 4 + i
                    # destination: slot0 cols 0:64 and slot1 cols 64:128
                    # (flat offsets 0:64 and 192:256 within this tile)
                    d0 = vslots[:, tt, 0, 0:HD]
                    dst = bass.AP(
                        tensor=d0.tensor,
                        offset=d0.offset,
                        ap=[list(d0.ap[0]), [192, 2], list(d0.ap[1])],
                    )
                    nc.vector.tensor_copy(
                        dst, tp.rearrange("p (g f) -> p g f", g=2)
                    )
                for wsb, bsb, dests in ((wk_sb, bk_sb, KT), (wq_sb, bq_sb, QT)):
                    ps = mix_ps.tile([128, 512], F32, name="mix_psum")
                    for kc in range(KC):
                        nc.tensor.matmul(
                            ps,
                            wsb[:, kc, :],
                            xts[kc],
                            start=(kc == 0),
                            stop=(kc == KC - 1),
                        )
                    nc.scalar.activation(
                        dests[:, ts(tcn, 512)], ps, AF.Identity, bias=bsb
                    )

            def outproj(g0):
                for mt in range(D // 128):
                    ps = mix_ps.tile([128, 512], F32, name="mix_psum")
                    nc.tensor.matmul(
                        ps,
                        wo_sb[:, ts(mt, 128)],
                        AT[:, g0 : g0 + 512],
                        start=True,
                        stop=True,
                    )
                    yt = yt_pool.tile([128, 512], BF, name="yt")
                    if mt == D // 128 - 1:
                        nc.scalar.activation(yt, ps, AF.Copy)
                    else:
                        nc.vector.tensor_copy(yt, ps)
                    nc.sync.dma_start(out=yT[ts(mt, 128), g0 : g0 + 512], in_=yt)

            pending = [None]

            def emit_attn(b, qc):
                g0 = b * S + qc * 512
                nkt = 4 * (qc + 1)  # causal: number of 128-key tiles
                acc0 = acc_ps.tile([128, 512], F32, name="accp", tag="accp")
                acc1 = acc_ps.tile([128, 512], F32, name="accp", tag="accp")
                accs = (acc0, acc1)
                # process the masked diagonal tiles first: their longer
                # exp -> mask -> PV chains overlap the unmasked tiles that
                # follow instead of delaying the normalize chain at the tail
                kt_order = list(range(4 * qc, nkt)) + list(range(0, 4 * qc))
                for ktpos, kt in enumerate(kt_order):
                    k0 = b * S + kt * 128
                    st = st_ps.tile([128, 1024], F32, name="st_psum")
                    # k=64 per head; base partitions 0/64 auto-derive PE
                    # row-group tile positions, so the two run concurrently
                    for hl in range(2):
                        hs = slice(hl * HD, (hl + 1) * HD)
                        nc.tensor.matmul(
                            st[:, ts(hl, 512)],
                            KT[hs, k0 : k0 + 128],
                            QT[hs, g0 : g0 + 512],
                            start=True,
                            stop=True,
                        )
                    pt = pt_pool.tile([128, 1024], BF)
                    nc.scalar.activation(pt, st, AF.Exp, scale=0.125)
                    d = kt - 4 * qc
                    if d >= 0:
                        m = mask_sb[:, d, :]
                        m2 = bass.AP(
                            tensor=m.tensor,
                            offset=m.offset,
                            ap=[list(m.ap[0]), [0, 2], list(m.ap[1])],
                        )
                        nc.vector.tensor_mul(
                            pt.rearrange("p (h j) -> p h j", h=2),
                            pt.rearrange("p (h j) -> p h j", h=2),
                            m2,
                        )
                    for hl in range(HPC):
                        nc.tensor.matmul(
                            accs[hl],
                            vslots[:, b * (S // 128) + kt, hl, :],
                            pt[:, ts(hl, 512)],
                            start=(ktpos == 0),
                            stop=(ktpos == nkt - 1),
                        )
                # normalize: gather both rowsums into SBUF, broadcast them
                # to all 128 partitions with one selector matmul, then one
                # full-width reciprocal (the custom DVE op only honors
                # base partition 0, so recip runs on the broadcast)
                nc.vector.tensor_copy(rsrc[HD : HD + 1, :], acc0[HD : HD + 1, :])
                nc.vector.tensor_copy(rsrc[0:1, :], acc1[0:1, :])
                rmat_ps = mix_ps.tile([128, 512], F32, name="mix_psum")
                nc.tensor.matmul(rmat_ps, sel2, rsrc, start=True, stop=True)
                rmat = rmat_pool.tile([128, 512], F32)
                nc.vector.reciprocal_approx_fast(out=rmat, in_=rmat_ps)
                nc.vector.tensor_mul(
                    AT[0:HD, g0 : g0 + 512], acc0[0:HD, :], rmat[0:HD, :]
                )
                nc.vector.tensor_mul(
                    AT[HD:128, g0 : g0 + 512], acc1[HD:128, :], rmat[HD:128, :]
                )
                # partial output projection, deferred one chunk so the
                # tensor engine never drains while the normalize chain runs
                if pending[0] is not None:
                    outproj(pending[0])
                pending[0] = g0

            # interleave: projections are PE-bound, attention is exp-bound;
            # alternating them keeps both engines fed. A(b,qc) needs
            # projection chunk b*4+qc, emitted one step ahead.
            # order: P0 P1 A00 P2 A01 P3 A02 P4 A03 P5 A10 P6 A11 P7 A12 A13
            emit_proj(0)
            emit_proj(1)
            for step in range(B * QCH):
                b, qc = divmod(step, QCH)
                emit_attn(b, qc)
                if step + 2 < TCH:
                    emit_proj(step + 2)
            outproj(pending[0])

    nc.finalize()
    return nc


def _install_ntff_hook():
    """bass_utils' trace path needs antenv.axon_hooks, which this image's
    antenv lacks; synthesize it from the boot helper so NTFF profiling works."""
    try:
        from antenv.axon_hooks import get_axon_ntff_profile_hook  # noqa: F401

        return
    except ImportError:
        pass
    try:
        import types

        import antenv
        from trn_agent_boot.trn_boot import _ntff_profile_via_ctypes

        hook = _ntff_profile_via_ctypes("/opt/axon/libaxon_pjrt.so")
        mod = types.ModuleType("antenv.axon_hooks")
        state = {"hook": hook}
        mod.get_axon_ntff_profile_hook = lambda: state["hook"]
        mod.set_axon_ntff_profile_hook = lambda h: state.update(hook=h)
        sys.modules["antenv.axon_hooks"] = mod
        antenv.axon_hooks = mod
    except Exception:
        pass


_NC_CACHE: dict[str, bass.Bass] = {}


def _get_nc() -> bass.Bass:
    if "nc" not in _NC_CACHE:
        _NC_CACHE["nc"] = build_nc()
    return _NC_CACHE["nc"]


def _shard_inputs(inputs, Wq, bq, Wk, bk, Wv, bv, Wo, bo):
    x = np.ascontiguousarray(np.asarray(inputs, dtype=np.float32)).reshape(T, D)
    # xT[p, kc, t] = x[t, kc*128+p]
    xTh = np.ascontiguousarray(
        x.reshape(T, KC, 128).transpose(2, 1, 0).astype(BF_NP)
    )

    maskh = np.zeros((128, 4, 512), dtype=BF_NP)
    p = np.arange(128)[:, None]
    jj = np.arange(512)[None, :]
    for d in range(4):
        maskh[:, d, :] = (d * 128 + p <= jj).astype(BF_NP)

    def wslice(W, c):
        Wc = np.asarray(W, dtype=np.float32)[:, c * FPC : (c + 1) * FPC]
        # [128, KC, FPC] with [p, kc, m] = W[kc*128+p, m]
        return np.ascontiguousarray(
            Wc.reshape(KC, 128, FPC).transpose(1, 0, 2).astype(BF_NP)
        )

    in_maps = []
    for c in range(NCORES):
        in_maps.append(
            {
                "xT": xTh,
                "wq": wslice(Wq, c),
                "wk": wslice(Wk, c),
                "wv": wslice(Wv, c),
                "wo": np.ascontiguousarray(
                    np.asarray(Wo, dtype=np.float32)[
                        c * FPC : (c + 1) * FPC, :
                    ].astype(BF_NP)
                ),
                "bq": np.asarray(bq, np.float32)[c * FPC : (c + 1) * FPC, None],
                "bk": np.asarray(bk, np.float32)[c * FPC : (c + 1) * FPC, None],
                "bv": np.asarray(bv, np.float32)[c * FPC : (c + 1) * FPC, None],
                "maskT": maskh,
            }
        )
    return in_maps


def run_with_results(
    inputs,
    Wq,
    bq,
    Wk,
    bk,
    Wv,
    bv,
    Wo,
    bo,
    trace: bool = False,
):
    in_maps = _shard_inputs(inputs, Wq, bq, Wk, bk, Wv, bv, Wo, bo)
    if trace:
        _install_ntff_hook()
    nc = _get_nc()
    res = run_bass_kernel_spmd(
        nc, in_maps, core_ids=list(range(NCORES)), trace=trace
    )
    acc = np.zeros((D, T), dtype=np.float32)
    for c in range(NCORES):
        acc += res.results[c]["yT"].astype(np.float32)
    y = acc.T + np.asarray(bo, np.float32)[None, :]
    out = np.ascontiguousarray(y.reshape(B, S, D).astype(np.float32))
    return out, res


def kernel(**inputs) -> np.ndarray:
    out, _ = run_with_results(**inputs)
    return out


if __name__ == "__main__":
    nc = build_nc()
    print("built ok")
